# revision 26
# baseline (speedup 1.0000x reference)
"""Trainium2 Bass kernel for geodesic convolution (gnn_message_passing).

Reference computation (per mesh vertex m, M=50000, n_in=n_out=32, grid 5x8):
  1. pullback[m,k,:] = sum_t bc_weights[m,k,t] * signal[bc_indices[m,k,t],:]
  2. x_grid[m,b,:]   = sum_{k: rad*8+ang==b} pullback[m,k,:]
  3. out_pre[m,r,o]  = sum_{i,j,n} x_grid[m,(i,j),n] * kernel[i,(j+r)%8,o,n]
  4. out[m,o]        = max_r relu(out_pre[m,r,o])

Data-parallel over m on 8 cores (6272 padded rows each, 49 tiles of 128
vertices), raw-bass pipeline (manual semaphores; the Ant dma_gather ucode is
incompatible with TileContext's event-semaphore machinery). Per tile:
  - 15 `dma_gather`s (SWDGE ucode, mlp library) of 1024 idxs each fetch, per
    (vertex, slot), the 128-byte bf16 row-PAIR holding the indexed signal row
    (pair id = idx>>1 fits the ucode's int16 index limit; the wanted half is
    selected by folding idx&1 into per-half duplicated barycentric weights).
    1024 idxs (65 descs/DMA-engine) is the largest batch the HW SWDGE ring
    carveout takes; the gathers round-robin the 4 SWDGE queues so descriptor
    generation overlaps across Q7 CPU pairs (measured 2.6 ns/idx vs 9.7
    single-queue — desc-gen is THE bottleneck of this kernel).
  - DVE: multiply by the (slot, half) weights broadcast over 32 channels,
    then reduce the two halves into a (t, bin, n)-ordered f32 tile.
  - PE: per 128-wide contraction chunk, 3 PSUM-accumulated transpose-matmuls
    sum the barycentric t-slices while transposing (m,(b,n)) -> ((b,n),m);
    then a 10-chunk K-accumulated matmul against the precomputed
    rotated-kernel matrix W2 (1280, 256).
  - ACT: PSUM->SBUF copies and relu; DVE: max over the 8 rotations.

The grid scatter (step 2) is folded into the gather ordering: slots are
binned host-side by their (rad, ang) cell; round p gathers the p-th slot of
every bin (dummies gather pair 0 with weight 0). For the reference's meshgrid
rad/ang layout this is the identity ordering and R=1 (fast path). R>1 uses a
slower DVE-reduction path.
"""

import os
from contextlib import ExitStack

import numpy as np

import concourse.bacc as bacc
import concourse.bass as bass
import concourse.mybir as mybir
from concourse.bass_utils import run_bass_kernel_spmd

M, N_IN, N_OUT = 50000, 32, 32
N_RHO, N_THETA = 5, 8
KV = N_RHO * N_THETA            # 40 grid bins
NS = KV * 3                     # 120 gather slots per vertex
N_CORES = 8
TILE_M = 128
TILES_PER_CORE = 49             # 49*128 = 6272 >= ceil(50000/8)
M_CORE = TILES_PER_CORE * TILE_M
M_PAD = N_CORES * M_CORE        # 50176
NCHUNK = (KV * N_IN) // 128     # 10 contraction chunks of 128
ROT_OUT = N_THETA * N_OUT       # 256
NIDX = TILE_M * NS              # 15360 gather indices per tile
IDXF = NIDX // 16               # 960 idx free-dim (16-partition wrap)

f32 = mybir.dt.float32
bf16 = mybir.dt.bfloat16
i16 = mybir.dt.int16

last_exec_time_ns = None
last_result = None

_program_cache = {}


def _dma_gather_128(gp, out_ap, in_ap, idxs_ap, num_idxs, elem_size,
                    elem_step, queue_num=0, num_idxs_reg=None):
    """dma_gather with a 128-byte element on a 256-byte row stride.

    bass's dma_gather asserts elem_size_bytes % 256 == 0, but the ucode's
    non-transpose path only needs the TABLE ROW STRIDE (elem_step bytes) to be
    a 256B multiple (stride_bytes_256 encoding); the element itself is one
    descriptor of arbitrary length. Replicates the bass lowering minus that
    assert for the DRAM-source non-transpose case.
    """
    assert idxs_ap.dtype == mybir.dt.int16
    assert in_ap.dtype == out_ap.dtype
    elem_size_bytes = elem_size * mybir.dt.size(in_ap.dtype)
    assert elem_size_bytes > 0 and elem_size_bytes % 128 == 0
    assert in_ap.ap[-1][1] == out_ap.ap[-1][1] == elem_size
    assert out_ap.ap[0][1] * out_ap.ap[1][1] == ((num_idxs + 127) // 128) * 128
    assert in_ap.ap[0][0] == elem_step
    stride_bytes = elem_step * mybir.dt.size(in_ap.dtype)
    assert stride_bytes % 256 == 0 and stride_bytes // 256 < 256
    _in_ap = gp.lower_ap_dma(in_ap, for_custom_bir_dma=True)
    _idxs_ap = gp.lower_ap(idxs_ap)
    _out_ap = gp.lower_ap(out_ap)
    return gp.add_instruction(
        mybir.InstDMAGatherAnt(
            name=gp.bass.get_next_instruction_name(),
            ins=[*_in_ap, _idxs_ap,
                 gp.lower_val_access(gp.to_reg(num_idxs) if num_idxs_reg is None
                                     else num_idxs_reg)],
            outs=[_out_ap],
            transpose=False,
            num_idxs=num_idxs,
            elem_size=elem_size,
            stride_bytes_256=stride_bytes // 256,
            gen_mode=0,
            single_packet=True,
            queue_num=queue_num,
            sbuf_tokens_per_rank=0,
            sbuf_free_dim_per_rank=0,
            sbuf_free_dim_pad_per_rank=0,
            sbuf_byte_offset=0,
        ))


def _build_program(n_rounds: int, n_tiles: int):
    nc = bacc.Bacc("TRN2", target_bir_lowering=False, debug=False,
                   num_devices=N_CORES, dynamic_dma_scratch_size=16384,
                   num_swdge_queues=4)

    R = n_rounds
    fast = (R == 1)
    # bf16 pair rows padded to a 256B stride (gather elem 128B, step 256B)
    sig_d = nc.dram_tensor("signal", [M // 2, 4 * N_IN], bf16, kind="ExternalInput")
    idx_d = nc.dram_tensor("idx", [R, n_tiles, 128, IDXF], i16, kind="ExternalInput")
    wts_d = nc.dram_tensor("wts", [R, n_tiles, TILE_M, 2 * NS], bf16,
                           kind="ExternalInput")
    w2_d = nc.dram_tensor("w2", [128, NCHUNK, ROT_OUT], bf16, kind="ExternalInput")
    ident_d = nc.dram_tensor("identity", [128, 128], f32, kind="ExternalInput")
    out_d = nc.dram_tensor("out", [n_tiles, TILE_M, N_OUT], f32,
                           kind="ExternalOutput")

    Ns = n_tiles * R  # gather steps

    with ExitStack() as ctx:
        e = ctx.enter_context

        def sb(name, shape, dt=f32):
            return e(nc.sbuf_tensor(name, shape, dt))

        gbuf = [sb(f"g{i}", [TILE_M, NS, 2 * N_IN], bf16) for i in range(2)]
        idxb = [sb(f"idx{i}", [128, IDXF], i16) for i in range(2)]
        wtsb = [sb(f"wts{i}", [TILE_M, 2 * NS], bf16) for i in range(2)]
        if fast:
            phb = [sb(f"ph{i}", [TILE_M, 3, KV, N_IN]) for i in range(2)]
        else:
            phb = [sb(f"xg{i}", [TILE_M, KV, N_IN]) for i in range(2)]
        xtb = [sb(f"xt{i}", [128, NCHUNK, 128], bf16) for i in range(2)]
        rtb = [sb(f"rt{i}", [TILE_M, N_THETA, N_OUT]) for i in range(2)]
        w2sb = sb("w2sb", [128, NCHUNK, ROT_OUT], bf16)
        ident = sb("ident", [128, 128])
        pstb = [e(nc.psum_tensor(f"pst{i}", [128, 128], f32)) for i in range(2)]
        opsb = [e(nc.psum_tensor(f"ops{i}", [TILE_M, ROT_OUT], f32))
                for i in range(2)]

        block = e(nc.Block())
        s_idx = [e(nc.semaphore(f"s_idx{i}")) for i in range(2)]
        s_wts = [e(nc.semaphore(f"s_wts{i}")) for i in range(2)]
        s_g = [e(nc.semaphore(f"s_g{i}")) for i in range(2)]
        s_gb = [e(nc.semaphore(f"s_gb{i}")) for i in range(2)]
        s_out = [e(nc.semaphore(f"s_out{i}")) for i in range(2)]
        s_mult = e(nc.semaphore("s_mult"))
        s_red = e(nc.semaphore("s_red"))
        s_tp = e(nc.semaphore("s_tp"))
        s_xt = e(nc.semaphore("s_xt"))
        s_mm = e(nc.semaphore("s_mm"))
        s_relu = e(nc.semaphore("s_relu"))
        s_max = e(nc.semaphore("s_max"))
        s_w2 = e(nc.semaphore("s_w2"))
        s_id = e(nc.semaphore("s_id"))

        def w(eng, sem, val):
            if val > 0:
                eng.wait_ge(sem, val)

        def emit_max(tm, dv):
            rt = rtb[tm % 2]
            nc.vector.tensor_tensor(out=rt[:, 0:4, :], in0=rt[:, 0:4, :],
                                    in1=rt[:, 4:8, :], op=mybir.AluOpType.max
                                    ).then_inc(s_max, 1)
            dv.wait_ge(s_max, 3 * tm + 1)
            nc.vector.tensor_tensor(out=rt[:, 0:2, :], in0=rt[:, 0:2, :],
                                    in1=rt[:, 2:4, :], op=mybir.AluOpType.max
                                    ).then_inc(s_max, 1)
            dv.wait_ge(s_max, 3 * tm + 2)
            nc.vector.tensor_tensor(out=rt[:, 0:1, :], in0=rt[:, 0:1, :],
                                    in1=rt[:, 1:2, :], op=mybir.AluOpType.max
                                    ).then_inc(s_max, 1)

        def emit_relu(tm):
            nc.scalar.activation(
                out=rtb[tm % 2][:],
                in_=opsb[tm % 2][:].rearrange("p (r o) -> p r o", o=N_OUT),
                func=mybir.ActivationFunctionType.Relu).then_inc(s_relu, 1)

        # ---- SP sequencer: input + output DMA ----
        @block.sync
        def _(sp):
            # tiny warm-up transfer absorbs the cold-start latency of the
            # HWDGE path before the first (latency-critical) idx tile DMA
            sp.dma_start(out=ident[0:1, 0:2], in_=ident_d[0:1, 0:2])
            for q in range(Ns):
                t, r = divmod(q, R)
                # idx buf q%2: gathers of step q-2 must have retired
                w(sp, s_g[q % 2], 128 * (q // 2))
                w(sp, s_gb[q % 2], 112 * (q // 2))
                sp.dma_start(out=idxb[q % 2][:], in_=idx_d[r, t]
                             ).then_inc(s_idx[q % 2], 16)
                # wts buf q%2: mult q-2 (both halves in fast path) done
                w(sp, s_mult, 2 * (q - 1) if fast else q - 1)
                sp.dma_start(out=wtsb[q % 2][:], in_=wts_d[r, t]
                             ).then_inc(s_wts[q % 2], 16)
                if q == 0:
                    # after the first idx/wts tiles so the gather pipeline
                    # starts as early as possible
                    sp.dma_start(out=w2sb[:], in_=w2_d[:]).then_inc(s_w2, 16)
                    sp.dma_start(out=ident[:], in_=ident_d[:]).then_inc(s_id, 16)
                if r == R - 1 and t >= 2:
                    t_o = t - 2
                    w(sp, s_max, 3 * (t_o + 1))
                    sp.dma_start(out=out_d[t_o], in_=rtb[t_o % 2][:, 0, :]
                                 ).then_inc(s_out[t_o % 2], 16)
            for t_o in (n_tiles - 2, n_tiles - 1):
                w(sp, s_max, 3 * (t_o + 1))
                sp.dma_start(out=out_d[t_o], in_=rtb[t_o % 2][:, 0, :]
                             ).then_inc(s_out[t_o % 2], 16)
            sp.wait_ge(s_out[0], 16 * ((n_tiles + 1) // 2))
            sp.wait_ge(s_out[1], 16 * (n_tiles // 2))

        # ---- Pool: identity init + gathers ----
        @block.gpsimd
        def _(gp):
            nreg = gp.to_reg(1024)  # shared num_idxs register for all gathers
            gq = 0                  # continuous queue round-robin counter
            for q in range(Ns):
                w(gp, s_idx[q % 2], 16 * (q // 2 + 1))
                # g buf q%2 free (reduce q-2 done)
                w(gp, s_red, 2 * (q - 1) if fast else 6 * (q - 1))
                # 15 sub-gathers of 1024 idxs (65 descs/engine): the largest
                # size that fits the HW SWDGE ring carveout (1536 hangs)
                # round-robin the 4 SWDGE queues: desc-gen for queue k runs
                # on Q7 CPU pair (2k, 2k+1); the engine pipelines dispatch so
                # up to 4 gathers' desc-gen overlaps (measured 3.3x). The
                # rotation is continuous across steps (15 % 4 != 0) so queue
                # load stays balanced.
                for j in range(15):
                    _dma_gather_128(gp, gbuf[q % 2][:, j * 8:(j + 1) * 8, :],
                                    sig_d[:, 0:2 * N_IN],
                                    idxb[q % 2][:, j * 64:(j + 1) * 64],
                                    1024, 2 * N_IN, 4 * N_IN,
                                    queue_num=gq % 4, num_idxs_reg=nreg
                                    ).then_inc(
                        s_g[q % 2] if j < 8 else s_gb[q % 2], 16)
                    gq += 1

        # ---- DVE: weight mult, reduce, rotation max ----
        @block.vector
        def _(dv):
            for q in range(Ns):
                t, r = divmod(q, R)
                g = gbuf[q % 2]
                if not fast:
                    w(dv, s_g[q % 2], 128 * (q // 2 + 1))
                    w(dv, s_gb[q % 2], 112 * (q // 2 + 1))
                w(dv, s_wts[q % 2], 16 * (q // 2 + 1))
                # WAR: g buf q%2 was read by reduce of step q-2
                if fast:
                    w(dv, s_red, 2 * (q - 1))
                else:
                    w(dv, s_red, 6 * (q - 1))
                gv = g[:].rearrange("p s (h n) -> p (s h) n", n=N_IN)
                wb = wtsb[q % 2][:].to_broadcast([TILE_M, 2 * NS, N_IN])
                g5 = g[:].rearrange("p (b t) (h n) -> p b t h n", t=3, n=N_IN)
                if fast:
                    # split halves: bins 0-19 (slots 0-59, gathers 0-7) start
                    # after 8 gathers so the PE's first 5 chunks overlap the
                    # tail gathers of the step
                    w(dv, s_g[q % 2], 128 * (q // 2 + 1))
                    nc.vector.tensor_tensor(out=gv[:, 0:NS, :],
                                            in0=gv[:, 0:NS, :],
                                            in1=wb[:, 0:NS, :],
                                            op=mybir.AluOpType.mult
                                            ).then_inc(s_mult, 1)
                    if r == 0:
                        w(dv, s_tp, NCHUNK * (t - 1))  # ph buf free
                    dv.wait_ge(s_mult, 2 * q + 1)
                    ph_a = phb[t % 2][:, :, 0:KV // 2, :].rearrange(
                        "p t b n -> p b t n")
                    nc.vector.tensor_tensor(
                        out=ph_a, in0=g5[:, 0:KV // 2, :, 0, :],
                        in1=g5[:, 0:KV // 2, :, 1, :],
                        op=mybir.AluOpType.add).then_inc(s_red, 1)
                    w(dv, s_gb[q % 2], 112 * (q // 2 + 1))
                    nc.vector.tensor_tensor(out=gv[:, NS:2 * NS, :],
                                            in0=gv[:, NS:2 * NS, :],
                                            in1=wb[:, NS:2 * NS, :],
                                            op=mybir.AluOpType.mult
                                            ).then_inc(s_mult, 1)
                    dv.wait_ge(s_mult, 2 * q + 2)
                    ph_b = phb[t % 2][:, :, KV // 2:KV, :].rearrange(
                        "p t b n -> p b t n")
                    nc.vector.tensor_tensor(
                        out=ph_b, in0=g5[:, KV // 2:KV, :, 0, :],
                        in1=g5[:, KV // 2:KV, :, 1, :],
                        op=mybir.AluOpType.add).then_inc(s_red, 1)
                else:
                    nc.vector.tensor_tensor(out=gv, in0=gv, in1=wb,
                                            op=mybir.AluOpType.mult
                                            ).then_inc(s_mult, 1)
                    dv.wait_ge(s_mult, q + 1)  # RAW: reduce reads mult output
                    if r == 0:
                        w(dv, s_tp, NCHUNK * (t - 1))  # xg buf free
                if not fast:
                    xg = phb[t % 2]
                    for j in range(6):
                        tt, hh = divmod(j, 2)
                        sl = g5[:, :, tt, hh, :]
                        if j >= 1:
                            dv.wait_ge(s_red, 6 * q + j)
                        elif r > 0:
                            dv.wait_ge(s_red, 6 * q)
                        if r == 0 and j == 0:
                            ins = nc.vector.tensor_copy(out=xg[:], in_=sl)
                        else:
                            ins = nc.vector.tensor_tensor(
                                out=xg[:], in0=xg[:], in1=sl,
                                op=mybir.AluOpType.add)
                        ins.then_inc(s_red, 1)
                if r == R - 1 and t >= 1:
                    tm = t - 1
                    w(dv, s_relu, tm + 1)
                    emit_max(tm, dv)
            tm = n_tiles - 1
            w(dv, s_relu, tm + 1)
            emit_max(tm, dv)

        # ---- PE: transpose-reduce + matmuls ----
        @block.tensor
        def _(pe):
            pe.wait_ge(s_id, 16)
            pe.wait_ge(s_w2, 16)
            for t in range(n_tiles):
                ph = phb[t % 2]
                if not fast:
                    # reduce of tile t fully done
                    w(pe, s_red, 6 * R * (t + 1))
                for c in range(NCHUNK):
                    if fast and c == 0:
                        w(pe, s_red, 2 * t + 1)   # bins 0-19 reduced
                    elif fast and c == NCHUNK // 2:
                        w(pe, s_red, 2 * t + 2)   # bins 20-39 reduced
                    G = NCHUNK * t + c
                    pst = pstb[G % 2]
                    w(pe, s_xt, G - 1)  # pst free: copy of chunk G-2 done
                    if fast:
                        for tt in range(3):
                            sl = ph[:, tt, c * 4:(c + 1) * 4, :]
                            ins = nc.tensor.matmul(
                                out=pst[:], lhsT=sl, rhs=ident[:],
                                is_transpose=True,
                                start=(tt == 0), stop=(tt == 2))
                    else:
                        xg2 = ph[:].rearrange("p k n -> p (k n)")
                        ins = nc.tensor.matmul(
                            out=pst[:], lhsT=xg2[:, c * 128:(c + 1) * 128],
                            rhs=ident[:], is_transpose=True,
                            start=True, stop=True)
                    ins.then_inc(s_tp, 1)
                w(pe, s_xt, NCHUNK * (t + 1))
                w(pe, s_relu, t - 1)  # ops buf free
                ops = opsb[t % 2]
                for c in range(NCHUNK):
                    ins = nc.tensor.matmul(out=ops[:], lhsT=xtb[t % 2][:, c, :],
                                           rhs=w2sb[:, c, :],
                                           start=(c == 0),
                                           stop=(c == NCHUNK - 1))
                    if c == NCHUNK - 1:
                        ins.then_inc(s_mm, 1)

        # ---- ACT: PSUM->SBUF copies + relu ----
        @block.scalar
        def _(ac):
            for t in range(n_tiles):
                for c in range(NCHUNK):
                    G = NCHUNK * t + c
                    w(ac, s_tp, G + 1)
                    w(ac, s_mm, t - 1)  # xt buf free
                    nc.scalar.copy(out=xtb[t % 2][:, c, :],
                                   in_=pstb[G % 2][:]).then_inc(s_xt, 1)
                if t >= 1:
                    tm = t - 1
                    w(ac, s_mm, tm + 1)
                    w(ac, s_out[tm % 2], 16 * (tm // 2))  # rt buf free
                    emit_relu(tm)
            tm = n_tiles - 1
            w(ac, s_mm, tm + 1)
            w(ac, s_out[tm % 2], 16 * (tm // 2))
            emit_relu(tm)

    nc.compile()
    return nc


def _build_w2(kernel):
    # W2[(i*8+j)*32+n, r*32+o] = kernel[i, (j+r)%8, o, n]
    k_rot = np.stack([np.roll(kernel, -r, axis=1) for r in range(N_THETA)], axis=0)
    w2 = k_rot.transpose(1, 2, 4, 0, 3).reshape(KV * N_IN, ROT_OUT)
    return np.ascontiguousarray(
        w2.reshape(NCHUNK, 128, ROT_OUT).transpose(1, 0, 2)).astype(np.float32)


def _build_rounds(bc_indices, bc_weights, rad_idx, ang_idx):
    flat = rad_idx.astype(np.int64) * N_THETA + ang_idx.astype(np.int64)
    if np.array_equal(flat, np.broadcast_to(np.arange(KV), flat.shape)):
        return (np.ascontiguousarray(bc_indices, dtype=np.int32)[None],
                np.ascontiguousarray(bc_weights, dtype=np.float32)[None])
    order = np.argsort(flat, axis=1, kind="stable")
    fs = np.take_along_axis(flat, order, axis=1)
    pos = np.broadcast_to(np.arange(KV), fs.shape)
    is_start = np.ones_like(fs, dtype=bool)
    is_start[:, 1:] = fs[:, 1:] != fs[:, :-1]
    start_pos = np.maximum.accumulate(np.where(is_start, pos, 0), axis=1)
    rank = (pos - start_pos).astype(np.int64)
    n_rounds = int(rank.max()) + 1
    bi_s = np.take_along_axis(bc_indices, order[:, :, None], axis=1)
    bw_s = np.take_along_axis(bc_weights, order[:, :, None], axis=1)
    m = flat.shape[0]
    gidx = np.zeros((n_rounds, m, KV, 3), dtype=np.int32)
    gw = np.zeros((n_rounds, m, KV, 3), dtype=np.float32)
    mm = np.broadcast_to(np.arange(m)[:, None], fs.shape)
    gidx[rank.ravel(), mm.ravel(), fs.ravel()] = bi_s.reshape(-1, 3)
    gw[rank.ravel(), mm.ravel(), fs.ravel()] = bw_s.reshape(-1, 3)
    return gidx, gw


def _prep_inputs(gidx, gw):
    """(R, M, KV, 3) idx/weights -> device idx16 (replicated 16-wrap) + dual-half
    weights: idx16 (n_cores, R, n_tiles, 128, IDXF) i16,
    wts (n_cores, R, n_tiles, 128, 240) f32."""
    n_rounds = gidx.shape[0]
    gidx_p = np.zeros((n_rounds, M_PAD, NS), dtype=np.int32)
    gw_p = np.zeros((n_rounds, M_PAD, NS), dtype=np.float32)
    gidx_p[:, :M] = gidx.reshape(n_rounds, M, NS)
    gw_p[:, :M] = gw.reshape(n_rounds, M, NS)

    pair = (gidx_p >> 1).astype(np.int16)
    half = (gidx_p & 1).astype(np.float32)
    wts = np.empty((n_rounds, M_PAD, NS, 2), dtype=np.float32)
    wts[..., 0] = gw_p * (1.0 - half)
    wts[..., 1] = gw_p * half
    wts = wts.reshape(n_rounds, N_CORES, TILES_PER_CORE, TILE_M, 2 * NS)
    wts = np.ascontiguousarray(wts.transpose(1, 0, 2, 3, 4))

    # gather order i = s*128 + m -> per-tile flat list (NS, 128)
    pair = pair.reshape(n_rounds, N_CORES, TILES_PER_CORE, TILE_M, NS)
    idx_flat = pair.transpose(1, 0, 2, 4, 3).reshape(
        N_CORES, n_rounds, TILES_PER_CORE, NIDX)
    # 16-partition wrap per 1024-idx sub-gather block, replicated to all
    # 8 groups of 16 partitions
    wrap = idx_flat.reshape(N_CORES, n_rounds, TILES_PER_CORE, 15, 64, 16)
    wrap = wrap.transpose(0, 1, 2, 5, 3, 4)  # (.., 16, 15, 64)
    wrap = wrap.reshape(N_CORES, n_rounds, TILES_PER_CORE, 16, IDXF)
    idx16 = np.ascontiguousarray(
        np.broadcast_to(wrap[:, :, :, None, :, :],
                        (N_CORES, n_rounds, TILES_PER_CORE, 8, 16, IDXF))
        .reshape(N_CORES, n_rounds, TILES_PER_CORE, 128, IDXF))
    return idx16, wts


def kernel(signal, kernel, bc_weights, bc_indices, rad_idx, ang_idx):
    global last_exec_time_ns, last_result
    import ml_dtypes
    signal = np.ascontiguousarray(np.asarray(signal), dtype=np.float32)
    sig_pairs = np.zeros((M // 2, 4 * N_IN), dtype=ml_dtypes.bfloat16)
    sig_pairs[:, :2 * N_IN] = signal.reshape(M // 2, 2 * N_IN)
    w2 = _build_w2(np.asarray(kernel, dtype=np.float32)).astype(ml_dtypes.bfloat16)
    gidx, gw = _build_rounds(np.asarray(bc_indices), np.asarray(bc_weights),
                             np.asarray(rad_idx), np.asarray(ang_idx))
    n_rounds = gidx.shape[0]
    idx16, wts = _prep_inputs(gidx, gw)

    key = (n_rounds, TILES_PER_CORE)
    if key not in _program_cache:
        _program_cache[key] = _build_program(n_rounds, TILES_PER_CORE)
    nc = _program_cache[key]

    ident = np.eye(128, dtype=np.float32)
    in_maps = [{"signal": sig_pairs, "idx": idx16[c],
                "wts": wts[c].astype(ml_dtypes.bfloat16), "w2": w2,
                "identity": ident}
               for c in range(N_CORES)]

    trace = bool(int(os.environ.get("BASS_KERNEL_TRACE", "0")))
    kwargs = {}
    if trace:
        try:
            import prof_shim
            prof_shim.install()
        except ImportError:
            pass
        tdir = os.environ.get("BASS_KERNEL_TRACE_DIR")
        if tdir:
            os.makedirs(tdir, exist_ok=True)
            kwargs["tmpdir"] = tdir
    res = run_bass_kernel_spmd(nc, in_maps, core_ids=list(range(N_CORES)),
                               trace=trace, **kwargs)
    last_result = res
    last_exec_time_ns = res.exec_time_ns

    out = np.concatenate([res.results[c]["out"].reshape(M_CORE, N_OUT)
                          for c in range(N_CORES)], axis=0)
    return np.ascontiguousarray(out[:M])

